# revision 11
# baseline (speedup 1.0000x reference)
"""Trainium2 Bass kernel for 16-head self-attention (D=1024, S=2048, B=2)
with upper-triangular (j >= i) mask and scale 1/head_dim.

Sharding: batch*head-group parallel over 8 cores. Core c handles batch
c//4, heads [4*(c%4), 4*(c%4)+4). Each core computes Q/K/V projections for
its 256 output dims, attention for its 4 heads, and a partial output
projection (its 256 rows of wo). Host sums the 4 partials per batch.

On-chip layout is transposed end-to-end: QT/KT [dh, seq], scores S^T
[seq_k, seq_q] (stationary=K^T chunk, moving=Q^T), exp on ScalarE
PSUM->SBUF with the 1/64 scale folded in, PV as O'^T = V'^T E^T with a
ones-column appended to V so row 64 of O' is the softmax denominator,
then out^T = wo^T O^T. The host transposes back.

v2 perf changes vs baseline:
 - removed a duplicated V-projection emission (was computed twice)
 - stationary-reuse: consecutive matmuls sharing lhsT skip the weight
   reload (InstMatmult.ldweights=False); loops reordered so Q/K-proj
   (k-outer/nb-inner), O-proj (c-outer/qb-inner) reuse stationaries
 - exp in 1024-col blocks (2-bank PSUM scores tiles) - fewer ACT instrs
 - softmax denom reciprocal via DVE reciprocal_approx_fast (~5x faster)
 - batched weight DMAs (one post per tensor), 4 DMA queues
 - output partials stored bf16 (halves output HBM traffic)
"""

import sys

sys.path.insert(0, "/opt/trn_rl_repo")

import numpy as np

import concourse.bass as bass
import concourse.mybir as mybir
from concourse import tile
from concourse.bass_utils import run_bass_kernel_spmd

# ---------------------------------------------------------------------------
# Workaround: this walrus build supports only 1 sync wait on the SP CTRL
# (drain) instruction; split the TileContext exit drain's waits across
# sequential drains (same-engine program order makes this equivalent).
_MAX_DRAIN_WAITS = 1


def _patched_drain_and_barrier(self, tick_clock, wait_clock):
    from bass_rust import ScopedClock

    nc = self.nc
    drain_inst = nc.sync.drain()
    wait_clock.add_sem_waits(
        drain_inst.ins, ScopedClock({None: tick_clock.global_clock})
    )
    si = drain_inst.ins.sync_info
    if si is not None and len(si.on_wait) > _MAX_DRAIN_WAITS:
        waits = list(si.on_wait)
        si.on_wait = waits[:_MAX_DRAIN_WAITS]
        rest = waits[_MAX_DRAIN_WAITS:]
        while rest:
            chunk, rest = rest[:_MAX_DRAIN_WAITS], rest[_MAX_DRAIN_WAITS:]
            extra = nc.sync.drain()
            esi = extra.ins.sync_info
            if esi is None:
                extra.ins.sync_info = mybir.SyncInfo(on_wait=chunk, on_update=[])
            else:
                esi.on_wait = chunk
    nc.all_engine_barrier()
    assert self.sems is not None
    popped = nc._tile_sem_poison_stack.pop()
    assert popped is self._sem_poison
    nc.clear_and_free_semaphores(list(self.sems.allocated().values()))
    nc.all_engine_barrier()


tile.TileContext._drain_and_barrier = _patched_drain_and_barrier


def _legalize_waits(nc, max_waits=1):
    """This walrus build accepts at most one sync wait per instruction.
    Hoist extra waits onto preceding NoOps on the same engine (same-engine
    program order preserves the gating semantics)."""
    for blk in nc.main_func.blocks:
        out = []
        for inst in blk.instructions:
            si = inst.sync_info
            if si is not None and len(si.on_wait) > max_waits:
                waits = list(si.on_wait)
                si.on_wait = waits[-max_waits:]
                for w in waits[:-max_waits]:
                    nop = mybir.InstNoOp(
                        name=nc.get_next_instruction_name(), ins=[], outs=[]
                    )
                    nop.engine = inst.engine
                    nop.sync_info = mybir.SyncInfo(on_wait=[w], on_update=[])
                    nc.register_instruction(nop)
                    out.append(nop)
            out.append(inst)
        blk.instructions[:] = out


# ---------------------------------------------------------------------------

B, S, D = 2, 2048, 1024
H, HD = 16, 64
SCALE = 1.0 / HD
NCORES = 8
HPC = 4          # heads per core
DHC = HPC * HD   # 256 head-dims per core
P = 128
KC = D // P      # 8 contraction chunks for projections
SC = S // P      # 16 seq chunks of 128
QB = 512         # seq_q block for PV / O-proj
NQB = S // QB    # 4
EB = 2 * QB      # exp block (2 PSUM banks)

F32 = mybir.dt.float32
F32R = mybir.dt.float32r
BF16 = mybir.dt.bfloat16

# toggles for bisection
USE_LDW_SKIP = True
USE_FAST_RECIP = True
OUT_BF16 = True

_COMPILED = None


def _mm(nc, out, lhsT, rhs, start, stop, reuse=False):
    m = nc.tensor.matmul(out, lhsT, rhs, start=start, stop=stop)
    if reuse and USE_LDW_SKIP:
        m.ins.ldweights = False
    return m


def _build_nc(loop_iters=None, phases="abc"):
    nc = bass.Bass("TRN2", target_bir_lowering=False, debug=False,
                   num_devices=NCORES)

    xT = nc.declare_dram_parameter("xT", [D, S], BF16, isOutput=False)
    wq = nc.declare_dram_parameter("wq", [D, DHC], BF16, isOutput=False)
    wk = nc.declare_dram_parameter("wk", [D, DHC], BF16, isOutput=False)
    wv = nc.declare_dram_parameter("wv", [D, DHC], BF16, isOutput=False)
    wo = nc.declare_dram_parameter("wo", [DHC, D], BF16, isOutput=False)
    bq = nc.declare_dram_parameter("bq", [2, P, 1], F32, isOutput=False)
    bk = nc.declare_dram_parameter("bk", [2, P, 1], F32, isOutput=False)
    bv = nc.declare_dram_parameter("bv", [P, DHC], F32, isOutput=False)
    tri = nc.declare_dram_parameter("tri", [P, P], BF16, isOutput=False)
    outT = nc.declare_dram_parameter("outT", [D, S], BF16 if OUT_BF16 else F32,
                                     isOutput=True)

    from contextlib import ExitStack
    with tile.TileContext(nc) as tc:
        _loop = ExitStack()
        if loop_iters:
            _loop.enter_context(tc.For_i(0, loop_iters, 1))
        dmaq = [nc.sync, nc.scalar, nc.gpsimd]
        dq = [0]

        def dma(out_ap, in_ap):
            eng = dmaq[dq[0] % len(dmaq)]
            dq[0] += 1
            return eng.dma_start(out_ap, in_ap)

        with (
            tc.tile_pool(name="persist", bufs=1) as pp,
            tc.tile_pool(name="stage", bufs=2) as stage,
            tc.tile_pool(name="epool", bufs=4) as epool,
            tc.tile_pool(name="small", bufs=4) as small,
        ):
            # ---------------- Phase A: load, cast, project ----------------
            # persistent bf16 tensors
            xTb = [pp.tile([P, S], BF16, tag=f"xtb{k}", name=f"xtb{k}") for k in range(KC)]
            # packed weight tiles: [128, k-chunk, DHC]
            wqb = pp.tile([P, KC * DHC], BF16, tag="wqb", name="wqb")
            wkb = pp.tile([P, KC * DHC], BF16, tag="wkb", name="wkb")
            wvb = pp.tile([P, KC * DHC], BF16, tag="wvb", name="wvb")
            wob = pp.tile([P, 2 * D], BF16, tag="wob", name="wob")
            QT = [pp.tile([P, S], BF16, tag=f"qt{m}", name=f"qt{m}") for m in range(2)]
            KT = [pp.tile([P, S], BF16, tag=f"kt{m}", name=f"kt{m}") for m in range(2)]
            # V with a ones column per head: [h0(64) 1 | h1(64) 1 | ...]
            Vb = [pp.tile([P, HPC * 65], BF16, tag=f"vb{s}", name=f"vb{s}") for s in range(SC)]
            OT = [pp.tile([P, S], BF16, tag=f"ot{m}", name=f"ot{m}") for m in range(2)]
            trib = pp.tile([P, P], BF16, tag="trib")
            bq_sb = pp.tile([P, 2], F32, tag="bq")
            bk_sb = pp.tile([P, 2], F32, tag="bk")
            bv_bc = pp.tile([P, DHC], F32, tag="bvbc")
            ones1 = pp.tile([1, 64], BF16, tag="ones1")

            def wq3(t):
                return t[:].rearrange("p (k c) -> p k c", k=KC)

            # batched weight posts + per-k xT posts, spread over 4 queues
            dma(wq3(wqb), wq[:, :].rearrange("(k p) c -> p k c", p=P))
            dma(xTb[0][:], xT[0:P, :])
            dma(wq3(wkb), wk[:, :].rearrange("(k p) c -> p k c", p=P))
            dma(xTb[1][:], xT[P:2 * P, :])
            dma(wq3(wvb), wv[:, :].rearrange("(k p) c -> p k c", p=P))
            for k in range(2, KC):
                dma(xTb[k][:], xT[k * P:(k + 1) * P, :])
            dma(trib[:], tri[:, :])
            nc.sync.dma_start(bq_sb[:, 0:1], bq[0])
            nc.sync.dma_start(bq_sb[:, 1:2], bq[1])
            nc.sync.dma_start(bk_sb[:, 0:1], bk[0])
            nc.sync.dma_start(bk_sb[:, 1:2], bk[1])
            nc.scalar.dma_start(bv_bc[:], bv[:, :])
            dma(wob[:].rearrange("p (c d) -> p c d", c=2),
                wo[:, :].rearrange("(c p) d -> p c d", p=P))
            nc.gpsimd.memset(ones1[:], 1.0)

            with tc.tile_pool(name="apsum", bufs=8, space="PSUM") as aps:
                # QT / KT: out [dh-chunk 128, seq]; lhsT = w chunk, rhs = xT.
                # k-outer / nb-inner with 4 live PSUM tiles so the stationary
                # (w chunk) is loaded once per k.
                proj_order = [(wqb, QT, bq_sb, 0), (wkb, KT, bk_sb, 0), None,
                              (wqb, QT, bq_sb, 1), (wkb, KT, bk_sb, 1)]
                for item in proj_order:
                    if item is None:
                        # V: out [seq chunk, 256]; lhsT = xT chunk, rhs = wv
                        for s in range(SC):
                            ps = aps.tile([P, QB], F32, tag="proj",
                                          name=f"vproj{s}")
                            for k in range(KC):
                                _mm(nc, ps[:, 0:DHC],
                                    xTb[k][:, s * P:(s + 1) * P],
                                    wq3(wvb)[:, k, :],
                                    start=(k == 0), stop=(k == KC - 1))
                            # scatter heads into 65-stride layout w/ bias add
                            vout = Vb[s][:].rearrange("p (h x) -> p h x", h=HPC)[:, :, 0:64]
                            psr = ps[:, 0:DHC].rearrange("p (h x) -> p h x", h=HPC)
                            bvr = bv_bc[:].rearrange("p (h x) -> p h x", h=HPC)
                            nc.vector.tensor_add(vout, psr, bvr)
                            ones = Vb[s][:].rearrange("p (h x) -> p h x", h=HPC)[:, :, 64:65]
                            nc.gpsimd.memset(ones, 1.0)
                        continue
                    (wb, dst, bias, m) = item
                    ps = [aps.tile([P, QB], F32, tag="proj", name=f"pj{m}{nb}")
                          for nb in range(NQB)]
                    for k in range(KC):
                        lhsT = wq3(wb)[:, k, m * P:(m + 1) * P]
                        for nb in range(NQB):
                            _mm(nc, ps[nb][:], lhsT,
                                xTb[k][:, nb * QB:(nb + 1) * QB],
                                start=(k == 0), stop=(k == KC - 1),
                                reuse=(nb > 0))
                    for nb in range(NQB):
                        nc.vector.tensor_scalar_add(
                            dst[m][:, nb * QB:(nb + 1) * QB],
                            ps[nb][:],
                            bias[:, m:m + 1],
                        )

            # ---------------- Phase B: attention per head ----------------
            if "b" in phases:
             with (
                tc.tile_pool(name="scpsum", bufs=2, space="PSUM") as scp,
                tc.tile_pool(name="opsum", bufs=1, space="PSUM") as opp,
            ):
                pending_norm = [None]

                def flush_norm():
                    if pending_norm[0] is not None:
                        pending_norm[0]()
                        pending_norm[0] = None

                for h in range(HPC):
                    m, poff = h // 2, 64 * (h % 2)
                    kt_h = KT[m][poff:poff + 64, :]
                    qt_h = QT[m][poff:poff + 64, :]
                    ops = opp.tile([65, S], F32, tag="oacc", name="oacc")
                    for jc in range(SC):
                        W = P * (jc + 1)
                        e = epool.tile([P, S], BF16, tag="e")
                        # zero-fill the tail up to the next 512 boundary
                        # first (read by PV, must be 0; no dep on exp)
                        zf = (QB - W % QB) % QB
                        if zf:
                            nc.gpsimd.memset(e[:, W:W + zf], 0.0)
                        # scores S^T[jc] = K_h^T[:,jc-chunk]^T . Q_h^T in
                        # 1024-col 2-bank psum tiles; exp consumes a whole
                        # tile (fewer ACT instrs). Stationary (kt chunk) is
                        # loaded once per jc.
                        first = True
                        for b0 in range(0, W, EB):
                            bw = min(EB, W - b0)
                            ps = scp.tile([P, EB], F32, tag="sc")
                            for ci in range(0, bw, QB):
                                cw = min(QB, bw - ci)
                                _mm(nc, ps[:, ci:ci + cw],
                                    kt_h[:, jc * P:(jc + 1) * P],
                                    qt_h[:, b0 + ci:b0 + ci + cw],
                                    start=True, stop=True, reuse=(not first))
                                first = False
                            nc.scalar.activation(
                                e[:, b0:b0 + bw],
                                ps[:, 0:bw],
                                mybir.ActivationFunctionType.Exp,
                                scale=SCALE,
                            )
                        # mask the diagonal 128-block post-exp (x0/1)
                        nc.gpsimd.tensor_mul(
                            e[:, W - P:W], e[:, W - P:W], trib[:]
                        )
                        # PV: accumulate O'^T[qb] over jc; stationary (V
                        # chunk) loaded once per jc
                        for qb in range((jc // 4) + 1):
                            _mm(nc, ops[:, qb * QB:(qb + 1) * QB],
                                Vb[jc][:, 65 * h:65 * h + 65],
                                e[:, qb * QB:(qb + 1) * QB],
                                start=(jc == 4 * qb), stop=(jc == SC - 1),
                                reuse=(qb > 0))
                    # evict O' to SBUF per qb-block (progressive bank
                    # release for the next head's PV), defer the normalize
                    # emission so the next head's PE stream isn't blocked
                    # behind the DVE recip chain
                    o_sb = small.tile([65, S], F32, tag="osb", bufs=2)
                    for qb in range(NQB):
                        nc.vector.tensor_copy(
                            o_sb[:, qb * QB:(qb + 1) * QB],
                            ops[:, qb * QB:(qb + 1) * QB])

                    def norm(m=m, poff=poff, o_sb=o_sb):
                        # 1/denom = exp(-ln(denom)) on ScalarE (measured
                        # 5e-5 rel err; DVE's reciprocal op costs 13us per
                        # head). bf16 result feeds a K=1 broadcast matmul.
                        lrow = small.tile([1, S], F32, tag="lrow", bufs=2)
                        rrow_b = small.tile([1, S], BF16, tag="rrow", bufs=2)
                        nc.scalar.activation(
                            lrow[:], o_sb[64:65, :],
                            mybir.ActivationFunctionType.Ln)
                        with nc.allow_low_precision(
                                reason="bf16 softmax denom broadcast"):
                            nc.scalar.activation(
                                rrow_b[:], lrow[:],
                                mybir.ActivationFunctionType.Exp,
                                scale=-1.0)
                        for qb in range(NQB):
                            rbp = scp.tile([64, EB], F32, tag="sc")
                            nc.tensor.matmul(
                                rbp[:, 0:QB], ones1[:],
                                rrow_b[:, qb * QB:(qb + 1) * QB],
                                start=True, stop=True,
                            )
                            nc.vector.tensor_mul(
                                OT[m][poff:poff + 64, qb * QB:(qb + 1) * QB],
                                o_sb[0:64, qb * QB:(qb + 1) * QB],
                                rbp[:, 0:QB],
                            )

                    flush_norm()
                    pending_norm[0] = norm
                flush_norm()

            # ---------------- Phase C: output projection ----------------
            if "c" in phases:
             with tc.tile_pool(name="cpsum", bufs=8, space="PSUM") as cps:
                for mo in range(D // P):
                    ot = stage.tile([P, S], BF16 if OUT_BF16 else F32,
                                    tag="outstage")
                    ps = [cps.tile([P, QB], F32, tag="oproj", name=f"op{qb}")
                          for qb in range(NQB)]
                    for c in range(2):
                        lhsT = wob[:].rearrange("p (c d) -> p c d", c=2)[
                            :, c, mo * P:(mo + 1) * P]
                        for qb in range(NQB):
                            _mm(nc, ps[qb][:], lhsT,
                                OT[c][:, qb * QB:(qb + 1) * QB],
                                start=(c == 0), stop=(c == 1),
                                reuse=(qb > 0))
                    for qb in range(NQB):
                        if qb % 2 == 0:
                            nc.vector.tensor_copy(
                                ot[:, qb * QB:(qb + 1) * QB], ps[qb][:])
                        else:
                            nc.scalar.copy(
                                ot[:, qb * QB:(qb + 1) * QB], ps[qb][:])
                    dma(outT[mo * P:(mo + 1) * P, :], ot[:])
        _loop.close()
    _legalize_waits(nc)
    return nc


def _get_nc():
    global _COMPILED
    if _COMPILED is None:
        _COMPILED = _build_nc()
    return _COMPILED


def _make_in_maps(x, wq, bq, wk, bk, wv, bv, wo, bo):
    import ml_dtypes
    bf16 = ml_dtypes.bfloat16
    tri = np.tril(np.ones((P, P), dtype=bf16))
    in_maps = []
    for c in range(NCORES):
        b, g = c // 4, c % 4
        cols = slice(DHC * g, DHC * (g + 1))
        in_maps.append({
            "xT": np.ascontiguousarray(x[b].T).astype(bf16),
            "wq": np.ascontiguousarray(wq[:, cols]).astype(bf16),
            "wk": np.ascontiguousarray(wk[:, cols]).astype(bf16),
            "wv": np.ascontiguousarray(wv[:, cols]).astype(bf16),
            "wo": np.ascontiguousarray(wo[cols, :]).astype(bf16),
            "bq": np.ascontiguousarray(bq[cols]).reshape(2, P, 1),
            "bk": np.ascontiguousarray(bk[cols]).reshape(2, P, 1),
            "bv": np.ascontiguousarray(np.broadcast_to(bv[cols].reshape(1, DHC), (P, DHC))),
            "tri": tri,
        })
    return in_maps


def kernel(x, wq, bq, wk, bk, wv, bv, wo, bo, _trace=False, _trace_kwargs=None):
    x = np.asarray(x, dtype=np.float32)
    assert x.shape == (B, S, D), x.shape
    nc = _get_nc()
    in_maps = _make_in_maps(
        x, np.asarray(wq), np.asarray(bq), np.asarray(wk), np.asarray(bk),
        np.asarray(wv), np.asarray(bv), np.asarray(wo), np.asarray(bo))
    kw = {}
    if _trace:
        kw = dict(trace=True, **(_trace_kwargs or {}))
    res = run_bass_kernel_spmd(nc, in_maps, list(range(NCORES)), **kw)
    out = np.empty((B, S, D), dtype=np.float32)
    for b in range(B):
        acc = np.zeros((D, S), dtype=np.float32)
        for g in range(4):
            acc += np.asarray(res.results[4 * b + g]["outT"], dtype=np.float32)
        out[b] = acc.T + np.asarray(bo, dtype=np.float32)
    kernel.last_result = res
    return out


# revision 21
# speedup vs baseline: 1.0704x; 1.0704x over previous
"""Trainium2 Bass kernel for 16-head self-attention (D=1024, S=2048, B=2)
with upper-triangular (j >= i) mask and scale 1/head_dim.

Sharding: batch*head-group parallel over 8 cores. Core c handles batch
c//4, heads [4*(c%4), 4*(c%4)+4). Each core computes Q/K/V projections for
its 256 output dims, attention for its 4 heads, and a partial output
projection (its 256 rows of wo). Host sums the 4 partials per batch.

On-chip layout is transposed end-to-end: QT/KT [dh, seq], scores S^T
[seq_k, seq_q] (stationary=K^T chunk, moving=Q^T), exp on ScalarE
PSUM->SBUF with the 1/64 scale folded in, PV as O'^T = V'^T E^T with a
ones-column appended to V so row 64 of O' is the softmax denominator,
then out^T = wo^T O^T. The host transposes back.

v2 perf changes vs baseline:
 - removed a duplicated V-projection emission (was computed twice)
 - stationary-reuse: consecutive matmuls sharing lhsT skip the weight
   reload (InstMatmult.ldweights=False); loops reordered so Q/K-proj
   (k-outer/nb-inner), O-proj (c-outer/qb-inner) reuse stationaries
 - exp in 1024-col blocks (2-bank PSUM scores tiles) - fewer ACT instrs
 - softmax denom reciprocal via DVE reciprocal_approx_fast (~5x faster)
 - batched weight DMAs (one post per tensor), 4 DMA queues
 - output partials stored bf16 (halves output HBM traffic)
"""

import sys

sys.path.insert(0, "/opt/trn_rl_repo")

import numpy as np

import concourse.bass as bass
import concourse.mybir as mybir
from concourse import tile
from concourse.bass_utils import run_bass_kernel_spmd

# NOTE: walrus's --enable-ldw-opt=true (redundant LDWEIGHTS elimination) is
# incompatible with bass's move_matmul_waits_to_ldweights pass (standalone
# InstLdweights carrying waits make the opt pass reportError) - verified
# empirically, so every matmul keeps its ~104ns weight reload and the only
# way to cut that cost is emitting fewer, wider matmuls.

# ---------------------------------------------------------------------------
# Workaround: this walrus build supports only 1 sync wait on the SP CTRL
# (drain) instruction; split the TileContext exit drain's waits across
# sequential drains (same-engine program order makes this equivalent).
_MAX_DRAIN_WAITS = 1


def _patched_drain_and_barrier(self, tick_clock, wait_clock):
    from bass_rust import ScopedClock

    nc = self.nc
    drain_inst = nc.sync.drain()
    wait_clock.add_sem_waits(
        drain_inst.ins, ScopedClock({None: tick_clock.global_clock})
    )
    si = drain_inst.ins.sync_info
    if si is not None and len(si.on_wait) > _MAX_DRAIN_WAITS:
        waits = list(si.on_wait)
        si.on_wait = waits[:_MAX_DRAIN_WAITS]
        rest = waits[_MAX_DRAIN_WAITS:]
        while rest:
            chunk, rest = rest[:_MAX_DRAIN_WAITS], rest[_MAX_DRAIN_WAITS:]
            extra = nc.sync.drain()
            esi = extra.ins.sync_info
            if esi is None:
                extra.ins.sync_info = mybir.SyncInfo(on_wait=chunk, on_update=[])
            else:
                esi.on_wait = chunk
    nc.all_engine_barrier()
    assert self.sems is not None
    popped = nc._tile_sem_poison_stack.pop()
    assert popped is self._sem_poison
    nc.clear_and_free_semaphores(list(self.sems.allocated().values()))
    nc.all_engine_barrier()


tile.TileContext._drain_and_barrier = _patched_drain_and_barrier


def _legalize_waits(nc, max_waits=1):
    """This walrus build accepts at most one sync wait per instruction.
    Hoist extra waits onto preceding NoOps on the same engine (same-engine
    program order preserves the gating semantics)."""
    for blk in nc.main_func.blocks:
        out = []
        for inst in blk.instructions:
            si = inst.sync_info
            if si is not None and len(si.on_wait) > max_waits:
                waits = list(si.on_wait)
                si.on_wait = waits[-max_waits:]
                for w in waits[:-max_waits]:
                    nop = mybir.InstNoOp(
                        name=nc.get_next_instruction_name(), ins=[], outs=[]
                    )
                    nop.engine = inst.engine
                    nop.sync_info = mybir.SyncInfo(on_wait=[w], on_update=[])
                    nc.register_instruction(nop)
                    out.append(nop)
            out.append(inst)
        blk.instructions[:] = out


# ---------------------------------------------------------------------------

B, S, D = 2, 2048, 1024
H, HD = 16, 64
SCALE = 1.0 / HD
NCORES = 8
HPC = 4          # heads per core
DHC = HPC * HD   # 256 head-dims per core
P = 128
KC = D // P      # 8 contraction chunks for projections
SC = S // P      # 16 seq chunks of 128
QB = 512         # seq_q block for PV / O-proj
NQB = S // QB    # 4
EB = 2 * QB      # exp block (2 PSUM banks)

F32 = mybir.dt.float32
F32R = mybir.dt.float32r
BF16 = mybir.dt.bfloat16

# toggles for bisection
USE_LDW_SKIP = True
USE_FAST_RECIP = True
OUT_BF16 = True

_COMPILED = None


def _mm(nc, out, lhsT, rhs, start, stop, reuse=False, **kw):
    m = nc.tensor.matmul(out, lhsT, rhs, start=start, stop=stop, **kw)
    if reuse and USE_LDW_SKIP:
        m.ins.ldweights = False
    return m


def _build_nc(loop_iters=None, phases="abc"):
    nc = bass.Bass("TRN2", target_bir_lowering=False, debug=False,
                   num_devices=NCORES)

    xT = nc.declare_dram_parameter("xT", [D, S], BF16, isOutput=False)
    wq = nc.declare_dram_parameter("wq", [D, DHC], BF16, isOutput=False)
    wk = nc.declare_dram_parameter("wk", [D, DHC], BF16, isOutput=False)
    wv = nc.declare_dram_parameter("wv", [D, DHC], BF16, isOutput=False)
    wo = nc.declare_dram_parameter("wo", [DHC, D], BF16, isOutput=False)
    bq = nc.declare_dram_parameter("bq", [2, P, 1], F32, isOutput=False)
    bk = nc.declare_dram_parameter("bk", [2, P, 1], F32, isOutput=False)
    bv = nc.declare_dram_parameter("bv", [P, DHC], F32, isOutput=False)
    tri = nc.declare_dram_parameter("tri", [P, P], BF16, isOutput=False)
    outT = nc.declare_dram_parameter("outT", [D, S], BF16 if OUT_BF16 else F32,
                                     isOutput=True)

    from contextlib import ExitStack
    with tile.TileContext(nc) as tc:
        _loop = ExitStack()
        if loop_iters:
            _loop.enter_context(tc.For_i(0, loop_iters, 1))
        dmaq = [nc.sync, nc.scalar, nc.gpsimd]
        dq = [0]

        def dma(out_ap, in_ap):
            eng = dmaq[dq[0] % len(dmaq)]
            dq[0] += 1
            return eng.dma_start(out_ap, in_ap)

        with (
            tc.tile_pool(name="persist", bufs=1) as pp,
            tc.tile_pool(name="stage", bufs=2) as stage,
            tc.tile_pool(name="epool", bufs=4) as epool,
            tc.tile_pool(name="small", bufs=4) as small,
        ):
            # ---------------- Phase A: load, cast, project ----------------
            # persistent bf16 tensors
            xTb = [pp.tile([P, S], BF16, tag=f"xtb{k}", name=f"xtb{k}") for k in range(KC)]
            # packed weight tiles: [128, k-chunk, DHC]
            wqb = pp.tile([P, KC * DHC], BF16, tag="wqb", name="wqb")
            wkb = pp.tile([P, KC * DHC], BF16, tag="wkb", name="wkb")
            wvb = pp.tile([P, KC * DHC], BF16, tag="wvb", name="wvb")
            wob = pp.tile([P, 2 * D], BF16, tag="wob", name="wob")
            QT = [pp.tile([P, S], BF16, tag=f"qt{m}", name=f"qt{m}") for m in range(2)]
            KT = [pp.tile([P, S], BF16, tag=f"kt{m}", name=f"kt{m}") for m in range(2)]
            # V with a ones column per head: [h0(64) 1 | h1(64) 1 | ...]
            Vb = [pp.tile([P, HPC * 65], BF16, tag=f"vb{s}", name=f"vb{s}") for s in range(SC)]
            OT = [pp.tile([P, S], BF16, tag=f"ot{m}", name=f"ot{m}") for m in range(2)]
            trib = pp.tile([P, P], BF16, tag="trib")
            bq_sb = pp.tile([P, 2], F32, tag="bq")
            bk_sb = pp.tile([P, 2], F32, tag="bk")
            bv_bc = pp.tile([P, DHC], F32, tag="bvbc")
            ones1 = pp.tile([1, 64], BF16, tag="ones1")

            def wq3(t):
                return t[:].rearrange("p (k c) -> p k c", k=KC)

            # per-k posts interleaved across the 3 DMA queues so the first
            # projection's deps (wq chunk 0, xT chunk 0) land first
            for k in range(KC):
                dma(wq3(wqb)[:, k, :], wq[k * P:(k + 1) * P, :])
                dma(xTb[k][:], xT[k * P:(k + 1) * P, :])
                dma(wq3(wkb)[:, k, :], wk[k * P:(k + 1) * P, :])
                dma(wq3(wvb)[:, k, :], wv[k * P:(k + 1) * P, :])
            dma(trib[:], tri[:, :])
            nc.sync.dma_start(bq_sb[:, 0:1], bq[0])
            nc.sync.dma_start(bq_sb[:, 1:2], bq[1])
            nc.sync.dma_start(bk_sb[:, 0:1], bk[0])
            nc.sync.dma_start(bk_sb[:, 1:2], bk[1])
            nc.scalar.dma_start(bv_bc[:], bv[:, :])
            dma(wob[:].rearrange("p (c d) -> p c d", c=2),
                wo[:, :].rearrange("(c p) d -> p c d", p=P))
            nc.gpsimd.memset(ones1[:], 1.0)

            with tc.tile_pool(name="apsum", bufs=8, space="PSUM") as aps:
                # QT / KT: out [dh-chunk 128, seq]; lhsT = w chunk, rhs = xT.
                # k-outer / nb-inner with 4 live PSUM tiles so the stationary
                # (w chunk) is loaded once per k.
                proj_order = [(wqb, QT, bq_sb, 0), (wkb, KT, bk_sb, 0), None,
                              (wqb, QT, bq_sb, 1), (wkb, KT, bk_sb, 1)]
                for item in proj_order:
                    if item is None:
                        # V: out [seq chunk, 256]; lhsT = xT chunk, rhs = wv
                        for s in range(SC):
                            ps = aps.tile([P, QB], F32, tag="proj",
                                          name=f"vproj{s}")
                            for k in range(KC):
                                _mm(nc, ps[:, 0:DHC],
                                    xTb[k][:, s * P:(s + 1) * P],
                                    wq3(wvb)[:, k, :],
                                    start=(k == 0), stop=(k == KC - 1))
                            # scatter heads into 65-stride layout w/ bias add
                            vout = Vb[s][:].rearrange("p (h x) -> p h x", h=HPC)[:, :, 0:64]
                            psr = ps[:, 0:DHC].rearrange("p (h x) -> p h x", h=HPC)
                            bvr = bv_bc[:].rearrange("p (h x) -> p h x", h=HPC)
                            nc.vector.tensor_add(vout, psr, bvr)
                            ones = Vb[s][:].rearrange("p (h x) -> p h x", h=HPC)[:, :, 64:65]
                            nc.gpsimd.memset(ones, 1.0)
                        continue
                    (wb, dst, bias, m) = item
                    ps = [aps.tile([P, QB], F32, tag="proj", name=f"pj{m}{nb}")
                          for nb in range(NQB)]
                    for k in range(KC):
                        lhsT = wq3(wb)[:, k, m * P:(m + 1) * P]
                        for nb in range(NQB):
                            _mm(nc, ps[nb][:], lhsT,
                                xTb[k][:, nb * QB:(nb + 1) * QB],
                                start=(k == 0), stop=(k == KC - 1),
                                reuse=(nb > 0))
                    for nb in range(NQB):
                        nc.vector.tensor_scalar_add(
                            dst[m][:, nb * QB:(nb + 1) * QB],
                            ps[nb][:],
                            bias[:, m:m + 1],
                        )

            # ---------------- Phase B: attention per head ----------------
            if "b" in phases:
             with (
                tc.tile_pool(name="scpsum", bufs=4, space="PSUM") as scp,
                tc.tile_pool(name="opsum", bufs=1, space="PSUM") as opp,
            ):
                pending_norm = [None]

                def flush_norm():
                    if pending_norm[0] is not None:
                        pending_norm[0]()
                        pending_norm[0] = None

                for h in range(HPC):
                    m, poff = h // 2, 64 * (h % 2)
                    kt_h = KT[m][poff:poff + 64, :]
                    qt_h = QT[m][poff:poff + 64, :]
                    ops = opp.tile([65, S], F32, tag="oacc", name="oacc")
                    for jc in range(SC):
                        W = P * (jc + 1)
                        e = epool.tile([P, S], BF16, tag="e")
                        # scores S^T[jc] = K_h^T[:,jc-chunk]^T . Q_h^T in
                        # 512-col single-bank psum tiles. Stationary (kt
                        # chunk) is loaded once per jc.
                        first = True
                        for c0 in range(0, W, QB):
                            cw = min(QB, W - c0)
                            ps = scp.tile([P, QB], F32, tag="sc")
                            _mm(nc, ps[:, 0:cw],
                                kt_h[:, jc * P:(jc + 1) * P],
                                qt_h[:, c0:c0 + cw],
                                start=True, stop=True, reuse=(not first))
                            first = False
                            nc.scalar.activation(
                                e[:, c0:c0 + cw],
                                ps[:, 0:cw],
                                mybir.ActivationFunctionType.Exp,
                                scale=SCALE,
                            )
                        # mask the diagonal 128-block post-exp (x0/1)
                        nc.gpsimd.tensor_mul(
                            e[:, W - P:W], e[:, W - P:W], trib[:]
                        )
                        # PV: accumulate O'^T[qb] over jc (<=512-col matmul
                        # out is an ISA limit). The diagonal qb block is
                        # written at its exact width: start_tensor_calc
                        # zeroes the whole 2KB PSUM bank, so later wider
                        # accumulations read 0 in the tail - no e tail
                        # zero-fill and no wasted columns.
                        vh = Vb[jc][:, 65 * h:65 * h + 65]
                        last = (jc == SC - 1)
                        for qb in range((jc // 4) + 1):
                            cw = min(QB, W - qb * QB)
                            _mm(nc, ops[:, qb * QB:qb * QB + cw], vh,
                                e[:, qb * QB:qb * QB + cw],
                                start=(jc == 4 * qb), stop=last,
                                reuse=(qb > 0), skip_group_check=True)
                    # evict O' to SBUF per qb-block (progressive bank
                    # release for the next head's PV), defer the normalize
                    # emission so the next head's PE stream isn't blocked
                    # behind the DVE recip chain
                    o_sb = small.tile([65, S], F32, tag="osb", bufs=2)
                    for qb in range(NQB):
                        nc.vector.tensor_copy(
                            o_sb[:, qb * QB:(qb + 1) * QB],
                            ops[:, qb * QB:(qb + 1) * QB])

                    def norm(m=m, poff=poff, o_sb=o_sb):
                        # 1/denom = exp(-ln(denom)) on ScalarE (measured
                        # 5e-5 rel err; DVE's reciprocal op costs 13us per
                        # head). bf16 result feeds a K=1 broadcast matmul.
                        lrow = small.tile([1, S], F32, tag="lrow", bufs=2)
                        rrow_b = small.tile([1, S], BF16, tag="rrow", bufs=2)
                        nc.scalar.activation(
                            lrow[:], o_sb[64:65, :],
                            mybir.ActivationFunctionType.Ln)
                        with nc.allow_low_precision(
                                reason="bf16 softmax denom broadcast"):
                            nc.scalar.activation(
                                rrow_b[:], lrow[:],
                                mybir.ActivationFunctionType.Exp,
                                scale=-1.0)
                        for qb in range(NQB):
                            rbp = scp.tile([64, QB], F32, tag="sc")
                            nc.tensor.matmul(
                                rbp[:], ones1[:],
                                rrow_b[:, qb * QB:(qb + 1) * QB],
                                start=True, stop=True,
                            )
                            nc.vector.tensor_mul(
                                OT[m][poff:poff + 64, qb * QB:(qb + 1) * QB],
                                o_sb[0:64, qb * QB:(qb + 1) * QB],
                                rbp[:],
                            )

                    flush_norm()
                    pending_norm[0] = norm
                flush_norm()

            # ---------------- Phase C: output projection ----------------
            if "c" in phases:
             with tc.tile_pool(name="cpsum", bufs=8, space="PSUM") as cps:
                for mo in range(D // P):
                    ot = stage.tile([P, S], BF16 if OUT_BF16 else F32,
                                    tag="outstage")
                    ps = [cps.tile([P, QB], F32, tag="oproj", name=f"op{qb}")
                          for qb in range(NQB)]
                    for c in range(2):
                        lhsT = wob[:].rearrange("p (c d) -> p c d", c=2)[
                            :, c, mo * P:(mo + 1) * P]
                        for qb in range(NQB):
                            _mm(nc, ps[qb][:], lhsT,
                                OT[c][:, qb * QB:(qb + 1) * QB],
                                start=(c == 0), stop=(c == 1),
                                reuse=(qb > 0))
                    for qb in range(NQB):
                        if qb % 2 == 0:
                            nc.vector.tensor_copy(
                                ot[:, qb * QB:(qb + 1) * QB], ps[qb][:])
                        else:
                            nc.scalar.copy(
                                ot[:, qb * QB:(qb + 1) * QB], ps[qb][:])
                    dma(outT[mo * P:(mo + 1) * P, :], ot[:])
        _loop.close()
    _legalize_waits(nc)
    return nc


def _get_nc():
    global _COMPILED
    if _COMPILED is None:
        _COMPILED = _build_nc()
    return _COMPILED


def _make_in_maps(x, wq, bq, wk, bk, wv, bv, wo, bo):
    import ml_dtypes
    bf16 = ml_dtypes.bfloat16
    tri = np.tril(np.ones((P, P), dtype=bf16))
    in_maps = []
    for c in range(NCORES):
        b, g = c // 4, c % 4
        cols = slice(DHC * g, DHC * (g + 1))
        in_maps.append({
            "xT": np.ascontiguousarray(x[b].T).astype(bf16),
            "wq": np.ascontiguousarray(wq[:, cols]).astype(bf16),
            "wk": np.ascontiguousarray(wk[:, cols]).astype(bf16),
            "wv": np.ascontiguousarray(wv[:, cols]).astype(bf16),
            "wo": np.ascontiguousarray(wo[cols, :]).astype(bf16),
            "bq": np.ascontiguousarray(bq[cols]).reshape(2, P, 1),
            "bk": np.ascontiguousarray(bk[cols]).reshape(2, P, 1),
            "bv": np.ascontiguousarray(np.broadcast_to(bv[cols].reshape(1, DHC), (P, DHC))),
            "tri": tri,
        })
    return in_maps


def kernel(x, wq, bq, wk, bk, wv, bv, wo, bo, _trace=False, _trace_kwargs=None):
    x = np.asarray(x, dtype=np.float32)
    assert x.shape == (B, S, D), x.shape
    nc = _get_nc()
    in_maps = _make_in_maps(
        x, np.asarray(wq), np.asarray(bq), np.asarray(wk), np.asarray(bk),
        np.asarray(wv), np.asarray(bv), np.asarray(wo), np.asarray(bo))
    kw = {}
    if _trace:
        kw = dict(trace=True, **(_trace_kwargs or {}))
    res = run_bass_kernel_spmd(nc, in_maps, list(range(NCORES)), **kw)
    out = np.empty((B, S, D), dtype=np.float32)
    for b in range(B):
        acc = np.zeros((D, S), dtype=np.float32)
        for g in range(4):
            acc += np.asarray(res.results[4 * b + g]["outT"], dtype=np.float32)
        out[b] = acc.T + np.asarray(bo, dtype=np.float32)
    kernel.last_result = res
    return out


# revision 23
# speedup vs baseline: 1.1077x; 1.0348x over previous
"""Trainium2 Bass kernel for 16-head self-attention (D=1024, S=2048, B=2)
with upper-triangular (j >= i) mask and scale 1/head_dim.

Sharding: batch*head-group parallel over 8 cores. Core c handles batch
c//4, heads [4*(c%4), 4*(c%4)+4). Each core computes Q/K/V projections for
its 256 output dims, attention for its 4 heads, and a partial output
projection (its 256 rows of wo). Host sums the 4 partials per batch.

On-chip layout is transposed end-to-end: QT/KT [dh, seq], scores S^T
[seq_k, seq_q] (stationary=K^T chunk, moving=Q^T), exp on ScalarE
PSUM->SBUF with the 1/64 scale folded in, PV as O'^T = V'^T E^T with a
ones-column appended to V so row 64 of O' is the softmax denominator,
then out^T = wo^T O^T. The host transposes back.

Perf structure (the chip power-throttles to ~58% during attention, so
total switched work matters more than overlap alone):
 - Q/K projections in fp8 (e4m3) DoubleRow matmuls: 256-deep contraction
   at 0.5 cyc/col - 4 instead of 8 matmuls per PSUM tile and half the
   streamed bytes. fp8 error on q/k (~3%) only perturbs softmax scores
   by ~0.3% absolute - harmless (verified: rel err stays ~5e-3).
 - V projection stays bf16 (V feeds the output directly; fp8 there costs
   ~3% output error).
 - software-pipelined attention: PV pieces of chunk jc-1 are interleaved
   between the scores matmuls of chunk jc, so the PE never waits on the
   exp(ACT)+mask(gpsimd) chain.
 - PV writes its diagonal block at exact width; start_tensor_calc zeroes
   the whole 2KB PSUM bank so no e-tail zero-fill is needed.
 - softmax 1/denom = exp(-ln(d)) on ScalarE per 512-col chunk (DVE's
   reciprocal op costs ~6 cyc/elem; ACT pair is ~20x cheaper), chunked
   so the last head's normalize pipelines into the output projection.
 - output partials stored bf16 (halves output HBM traffic).
"""

import sys

sys.path.insert(0, "/opt/trn_rl_repo")

import numpy as np

import concourse.bass as bass
import concourse.mybir as mybir
from concourse import tile
from concourse.bass_utils import run_bass_kernel_spmd

# ---------------------------------------------------------------------------
# Workaround: this walrus build supports only 1 sync wait on the SP CTRL
# (drain) instruction; split the TileContext exit drain's waits across
# sequential drains (same-engine program order makes this equivalent).
_MAX_DRAIN_WAITS = 1


def _patched_drain_and_barrier(self, tick_clock, wait_clock):
    from bass_rust import ScopedClock

    nc = self.nc
    drain_inst = nc.sync.drain()
    wait_clock.add_sem_waits(
        drain_inst.ins, ScopedClock({None: tick_clock.global_clock})
    )
    si = drain_inst.ins.sync_info
    if si is not None and len(si.on_wait) > _MAX_DRAIN_WAITS:
        waits = list(si.on_wait)
        si.on_wait = waits[:_MAX_DRAIN_WAITS]
        rest = waits[_MAX_DRAIN_WAITS:]
        while rest:
            chunk, rest = rest[:_MAX_DRAIN_WAITS], rest[_MAX_DRAIN_WAITS:]
            extra = nc.sync.drain()
            esi = extra.ins.sync_info
            if esi is None:
                extra.ins.sync_info = mybir.SyncInfo(on_wait=chunk, on_update=[])
            else:
                esi.on_wait = chunk
    nc.all_engine_barrier()
    assert self.sems is not None
    popped = nc._tile_sem_poison_stack.pop()
    assert popped is self._sem_poison
    nc.clear_and_free_semaphores(list(self.sems.allocated().values()))
    nc.all_engine_barrier()


tile.TileContext._drain_and_barrier = _patched_drain_and_barrier


def _legalize_waits(nc, max_waits=1):
    """This walrus build accepts at most one sync wait per instruction.
    Hoist extra waits onto preceding NoOps on the same engine (same-engine
    program order preserves the gating semantics)."""
    for blk in nc.main_func.blocks:
        out = []
        for inst in blk.instructions:
            si = inst.sync_info
            if si is not None and len(si.on_wait) > max_waits:
                waits = list(si.on_wait)
                si.on_wait = waits[-max_waits:]
                for w in waits[:-max_waits]:
                    nop = mybir.InstNoOp(
                        name=nc.get_next_instruction_name(), ins=[], outs=[]
                    )
                    nop.engine = inst.engine
                    nop.sync_info = mybir.SyncInfo(on_wait=[w], on_update=[])
                    nc.register_instruction(nop)
                    out.append(nop)
            out.append(inst)
        blk.instructions[:] = out


# ---------------------------------------------------------------------------

B, S, D = 2, 2048, 1024
H, HD = 16, 64
SCALE = 1.0 / HD
NCORES = 8
HPC = 4          # heads per core
DHC = HPC * HD   # 256 head-dims per core
P = 128
KC = D // P      # 8 contraction chunks for projections
NSUP = KC // 2   # 4 fp8 DoubleRow super-chunks (256-deep each)
SC = S // P      # 16 seq chunks of 128
QB = 512         # seq_q block for PV / O-proj
NQB = S // QB    # 4

F32 = mybir.dt.float32
BF16 = mybir.dt.bfloat16
FP8 = mybir.dt.float8e4
DR = mybir.MatmulPerfMode.DoubleRow

_COMPILED = None


def _build_nc():
    nc = bass.Bass("TRN2", target_bir_lowering=False, debug=False,
                   num_devices=NCORES)

    xT = nc.declare_dram_parameter("xT", [D, S], BF16, isOutput=False)
    x8 = nc.declare_dram_parameter("x8", [D, S], FP8, isOutput=False)
    wq8 = nc.declare_dram_parameter("wq8", [D, DHC], FP8, isOutput=False)
    wk8 = nc.declare_dram_parameter("wk8", [D, DHC], FP8, isOutput=False)
    wv = nc.declare_dram_parameter("wv", [D, DHC], BF16, isOutput=False)
    wo = nc.declare_dram_parameter("wo", [DHC, D], BF16, isOutput=False)
    bq = nc.declare_dram_parameter("bq", [2, P, 1], F32, isOutput=False)
    bk = nc.declare_dram_parameter("bk", [2, P, 1], F32, isOutput=False)
    bv = nc.declare_dram_parameter("bv", [P, DHC], F32, isOutput=False)
    tri = nc.declare_dram_parameter("tri", [P, P], BF16, isOutput=False)
    outT = nc.declare_dram_parameter("outT", [D, S], BF16, isOutput=True)

    with tile.TileContext(nc) as tc:
        dmaq = [nc.sync, nc.scalar, nc.gpsimd]
        dq = [0]

        def dma(out_ap, in_ap):
            eng = dmaq[dq[0] % len(dmaq)]
            dq[0] += 1
            return eng.dma_start(out_ap, in_ap)

        with (
            tc.tile_pool(name="persist", bufs=1) as pp,
            tc.tile_pool(name="stage", bufs=2) as stage,
            tc.tile_pool(name="epool", bufs=4) as epool,
            tc.tile_pool(name="small", bufs=4) as small,
        ):
            # ---------------- Phase A: load, project ----------------
            xTb = [pp.tile([P, S], BF16, tag=f"xtb{k}", name=f"xtb{k}") for k in range(KC)]
            # fp8 moving operand for Q/K proj: per 256-deep super-chunk,
            # two 128-row planes side by side: [128, (plane, seq)]
            x8b = [pp.tile([P, 2 * S], FP8, tag=f"x8b{c}", name=f"x8b{c}")
                   for c in range(NSUP)]
            # fp8 stationary for Q/K proj, packed [128, (k, out-col)]
            wq8b = pp.tile([P, KC * DHC], FP8, tag="wq8b", name="wq8b")
            wk8b = pp.tile([P, KC * DHC], FP8, tag="wk8b", name="wk8b")
            wvb = pp.tile([P, KC * DHC], BF16, tag="wvb", name="wvb")
            wob = pp.tile([P, 2 * D], BF16, tag="wob", name="wob")
            QT = [pp.tile([P, S], BF16, tag=f"qt{m}", name=f"qt{m}") for m in range(2)]
            KT = [pp.tile([P, S], BF16, tag=f"kt{m}", name=f"kt{m}") for m in range(2)]
            # V with a ones column per head: [h0(64) 1 | h1(64) 1 | ...]
            Vb = [pp.tile([P, HPC * 65], BF16, tag=f"vb{s}", name=f"vb{s}") for s in range(SC)]
            OT = [pp.tile([P, S], BF16, tag=f"ot{m}", name=f"ot{m}") for m in range(2)]
            trib = pp.tile([P, P], BF16, tag="trib")
            bq_sb = pp.tile([P, 2], F32, tag="bq")
            bk_sb = pp.tile([P, 2], F32, tag="bk")
            bv_bc = pp.tile([P, DHC], F32, tag="bvbc")
            ones1 = pp.tile([1, 64], BF16, tag="ones1")

            def k3(t, width=DHC):
                return t[:].rearrange("p (k c) -> p k c", k=KC)

            def x83(c):
                return x8b[c][:].rearrange("p (two n) -> p two n", two=2)

            # DMA: super-chunk-major so the first Q-proj matmuls' deps
            # (wq8 supers, x8 supers) land first; 3 queues round-robin
            for c in range(NSUP):
                dma(k3(wq8b)[:, 2 * c:2 * c + 2, :],
                    wq8[2 * c * P:(2 * c + 2) * P, :]
                    .rearrange("(two p) n -> p two n", p=P))
                dma(x83(c),
                    x8[2 * c * P:(2 * c + 2) * P, :]
                    .rearrange("(two p) n -> p two n", p=P))
                dma(k3(wk8b)[:, 2 * c:2 * c + 2, :],
                    wk8[2 * c * P:(2 * c + 2) * P, :]
                    .rearrange("(two p) n -> p two n", p=P))
            for k in range(KC):
                dma(xTb[k][:], xT[k * P:(k + 1) * P, :])
                dma(k3(wvb)[:, k, :], wv[k * P:(k + 1) * P, :])
            dma(trib[:], tri[:, :])
            nc.sync.dma_start(bq_sb[:, 0:1], bq[0])
            nc.sync.dma_start(bq_sb[:, 1:2], bq[1])
            nc.sync.dma_start(bk_sb[:, 0:1], bk[0])
            nc.sync.dma_start(bk_sb[:, 1:2], bk[1])
            nc.scalar.dma_start(bv_bc[:], bv[:, :])
            dma(wob[:].rearrange("p (c d) -> p c d", c=2),
                wo[:, :].rearrange("(c p) d -> p c d", p=P))
            nc.gpsimd.memset(ones1[:], 1.0)

            with tc.tile_pool(name="apsum", bufs=8, space="PSUM") as aps:
                # QT / KT: out [dh-chunk 128, seq]; fp8 DoubleRow over
                # 256-deep super-chunks, super-outer / nb-inner
                proj_order = [(wq8b, QT, bq_sb, 0), (wk8b, KT, bk_sb, 0),
                              None,
                              (wq8b, QT, bq_sb, 1), (wk8b, KT, bk_sb, 1)]
                for item in proj_order:
                    if item is None:
                        # V: out [seq chunk, 256] bf16; lhsT = xT chunk
                        for s in range(SC):
                            ps = aps.tile([P, QB], F32, tag="proj",
                                          name=f"vproj{s}")
                            for k in range(KC):
                                nc.tensor.matmul(
                                    ps[:, 0:DHC],
                                    xTb[k][:, s * P:(s + 1) * P],
                                    k3(wvb)[:, k, :],
                                    start=(k == 0), stop=(k == KC - 1))
                            vout = Vb[s][:].rearrange("p (h x) -> p h x", h=HPC)[:, :, 0:64]
                            psr = ps[:, 0:DHC].rearrange("p (h x) -> p h x", h=HPC)
                            bvr = bv_bc[:].rearrange("p (h x) -> p h x", h=HPC)
                            nc.vector.tensor_add(vout, psr, bvr)
                            ones = Vb[s][:].rearrange("p (h x) -> p h x", h=HPC)[:, :, 64:65]
                            nc.gpsimd.memset(ones, 1.0)
                        continue
                    (w8b, dst, bias, m) = item
                    ps = [aps.tile([P, QB], F32, tag="proj", name=f"pj{m}{nb}")
                          for nb in range(NQB)]
                    for c in range(NSUP):
                        lhsT = (k3(w8b)[:, 2 * c:2 * c + 2, m * P:(m + 1) * P])
                        for nb in range(NQB):
                            nc.tensor.matmul(
                                ps[nb][:], lhsT,
                                x83(c)[:, :, nb * QB:(nb + 1) * QB],
                                start=(c == 0), stop=(c == NSUP - 1),
                                perf_mode=DR)
                    for nb in range(NQB):
                        nc.vector.tensor_scalar_add(
                            dst[m][:, nb * QB:(nb + 1) * QB],
                            ps[nb][:],
                            bias[:, m:m + 1],
                        )

            # ---------------- Phase B: attention per head ----------------
            with (
                tc.tile_pool(name="scpsum", bufs=4, space="PSUM") as scp,
                tc.tile_pool(name="opsum", bufs=1, space="PSUM") as opp,
            ):
                pending_norm = [None]

                def flush_norm():
                    if pending_norm[0] is not None:
                        pending_norm[0]()
                        pending_norm[0] = None

                for h in range(HPC):
                    m, poff = h // 2, 64 * (h % 2)
                    kt_h = KT[m][poff:poff + 64, :]
                    qt_h = QT[m][poff:poff + 64, :]
                    ops = opp.tile([65, S], F32, tag="oacc", name="oacc")

                    def pv_piece(jc, e, qb):
                        # exact-width diagonal write; start_tensor_calc
                        # zeroes the whole PSUM bank so later wider
                        # accumulations read 0 in the bank tail
                        W = P * (jc + 1)
                        cw = min(QB, W - qb * QB)
                        nc.tensor.matmul(
                            ops[:, qb * QB:qb * QB + cw],
                            Vb[jc][:, 65 * h:65 * h + 65],
                            e[:, qb * QB:qb * QB + cw],
                            start=(jc == 4 * qb), stop=(jc == SC - 1),
                            skip_group_check=True)

                    prev = None  # (jc, e) whose PV is pending
                    for jc in range(SC):
                        W = P * (jc + 1)
                        e = epool.tile([P, S], BF16, tag="e")
                        nch = (W + QB - 1) // QB
                        npc = (prev[0] // 4) + 1 if prev else 0
                        # interleave prev's PV pieces between this chunk's
                        # scores matmuls: the PE never waits on exp/mask
                        for i in range(max(nch, npc)):
                            if i < npc:
                                pv_piece(prev[0], prev[1], i)
                            if i < nch:
                                c0 = i * QB
                                cw = min(QB, W - c0)
                                ps = scp.tile([P, QB], F32, tag="sc")
                                nc.tensor.matmul(
                                    ps[:, 0:cw],
                                    kt_h[:, jc * P:(jc + 1) * P],
                                    qt_h[:, c0:c0 + cw],
                                    start=True, stop=True)
                                nc.scalar.activation(
                                    e[:, c0:c0 + cw],
                                    ps[:, 0:cw],
                                    mybir.ActivationFunctionType.Exp,
                                    scale=SCALE,
                                )
                        # mask the diagonal 128-block post-exp (x0/1)
                        nc.gpsimd.tensor_mul(
                            e[:, W - P:W], e[:, W - P:W], trib[:]
                        )
                        prev = (jc, e)
                    for qb in range(NQB):
                        pv_piece(prev[0], prev[1], qb)

                    # evict O' to SBUF per qb-block (progressive bank
                    # release), defer the normalize emission so the next
                    # head's PE stream isn't blocked
                    o_sb = small.tile([65, S], F32, tag="osb", bufs=2)
                    for qb in range(NQB):
                        nc.vector.tensor_copy(
                            o_sb[:, qb * QB:(qb + 1) * QB],
                            ops[:, qb * QB:(qb + 1) * QB])

                    def norm(m=m, poff=poff, o_sb=o_sb):
                        # per-qb 1/denom = exp(-ln d) on ScalarE (5e-5 rel
                        # err), K=1 bf16 broadcast matmul, multiply on DVE
                        lrow = small.tile([1, S], F32, tag="lrow", bufs=2)
                        rrow = small.tile([1, S], BF16, tag="rrow", bufs=2)
                        for qb in range(NQB):
                            sl = slice(qb * QB, (qb + 1) * QB)
                            nc.scalar.activation(
                                lrow[:, sl], o_sb[64:65, sl],
                                mybir.ActivationFunctionType.Ln)
                            with nc.allow_low_precision(
                                    reason="bf16 softmax denom broadcast"):
                                nc.scalar.activation(
                                    rrow[:, sl], lrow[:, sl],
                                    mybir.ActivationFunctionType.Exp,
                                    scale=-1.0)
                            rbp = scp.tile([64, QB], F32, tag="sc")
                            nc.tensor.matmul(
                                rbp[:], ones1[:], rrow[:, sl],
                                start=True, stop=True,
                            )
                            nc.vector.tensor_mul(
                                OT[m][poff:poff + 64, sl],
                                o_sb[0:64, sl],
                                rbp[:],
                            )

                    flush_norm()
                    pending_norm[0] = norm
                flush_norm()

            # ---------------- Phase C: output projection ----------------
            with tc.tile_pool(name="cpsum", bufs=8, space="PSUM") as cps:
                for mo in range(D // P):
                    ot = stage.tile([P, S], BF16, tag="outstage")
                    ps = [cps.tile([P, QB], F32, tag="oproj", name=f"op{qb}")
                          for qb in range(NQB)]
                    for c in range(2):
                        lhsT = wob[:].rearrange("p (c d) -> p c d", c=2)[
                            :, c, mo * P:(mo + 1) * P]
                        for qb in range(NQB):
                            nc.tensor.matmul(
                                ps[qb][:], lhsT,
                                OT[c][:, qb * QB:(qb + 1) * QB],
                                start=(c == 0), stop=(c == 1))
                    for qb in range(NQB):
                        if qb % 2 == 0:
                            nc.vector.tensor_copy(
                                ot[:, qb * QB:(qb + 1) * QB], ps[qb][:])
                        else:
                            nc.scalar.copy(
                                ot[:, qb * QB:(qb + 1) * QB], ps[qb][:])
                    dma(outT[mo * P:(mo + 1) * P, :], ot[:])
    _legalize_waits(nc)
    return nc


def _get_nc():
    global _COMPILED
    if _COMPILED is None:
        _COMPILED = _build_nc()
    return _COMPILED


def _make_in_maps(x, wq, bq, wk, bk, wv, bv, wo, bo):
    import ml_dtypes
    bf16 = ml_dtypes.bfloat16
    fp8 = ml_dtypes.float8_e4m3  # TRN fp8e4: max normal 240
    tri = np.tril(np.ones((P, P), dtype=bf16))
    in_maps = []
    for c in range(NCORES):
        b, g = c // 4, c % 4
        cols = slice(DHC * g, DHC * (g + 1))
        xt = np.ascontiguousarray(x[b].T)
        in_maps.append({
            "xT": xt.astype(bf16),
            "x8": xt.astype(fp8),
            "wq8": np.ascontiguousarray(wq[:, cols]).astype(fp8),
            "wk8": np.ascontiguousarray(wk[:, cols]).astype(fp8),
            "wv": np.ascontiguousarray(wv[:, cols]).astype(bf16),
            "wo": np.ascontiguousarray(wo[cols, :]).astype(bf16),
            "bq": np.ascontiguousarray(bq[cols]).reshape(2, P, 1),
            "bk": np.ascontiguousarray(bk[cols]).reshape(2, P, 1),
            "bv": np.ascontiguousarray(np.broadcast_to(bv[cols].reshape(1, DHC), (P, DHC))),
            "tri": tri,
        })
    return in_maps


def kernel(x, wq, bq, wk, bk, wv, bv, wo, bo, _trace=False, _trace_kwargs=None):
    x = np.asarray(x, dtype=np.float32)
    assert x.shape == (B, S, D), x.shape
    nc = _get_nc()
    in_maps = _make_in_maps(
        x, np.asarray(wq), np.asarray(bq), np.asarray(wk), np.asarray(bk),
        np.asarray(wv), np.asarray(bv), np.asarray(wo), np.asarray(bo))
    kw = {}
    if _trace:
        kw = dict(trace=True, **(_trace_kwargs or {}))
    res = run_bass_kernel_spmd(nc, in_maps, list(range(NCORES)), **kw)
    out = np.empty((B, S, D), dtype=np.float32)
    for b in range(B):
        acc = np.zeros((D, S), dtype=np.float32)
        for g in range(4):
            acc += np.asarray(res.results[4 * b + g]["outT"], dtype=np.float32)
        out[b] = acc.T + np.asarray(bo, dtype=np.float32)
    kernel.last_result = res
    return out
